# revision 29
# baseline (speedup 1.0000x reference)
"""Multi-head attention (B=8, N=1024, C=768, H=12, D=64) on 8 TRN2 NeuronCores.

Sharding: pure data parallel — one batch element per core, no collectives.

Per-core dataflow (all matmuls laid out so no on-device transposes are needed):
  A1: qT/kT feature-major:   qkvT[n,i] = sum_c wqkvT[c,n] * xT[c,i]
      (w stationary, xT moving).  q rows get *SCALE and +q_bias*SCALE on evict.
  A2: v token-major:         v[i,f] = sum_c xT[c,i] * wqkvT[c,f] + v_bias[f]
      (xT stationary, w moving; bias via ones-row matmul).  Evicted bf16 into
      v_aug layout [tok, h*65+d] with a ones column per head at d=64.
  B:  per head pair g: S.T[j,i] = kT_h[d,j].T @ qT_h[d,i]  (K=64, heads 2g /
      2g+1 in partition rows 0:64 / 64:128 -> concurrent row-group matmuls).
      exp on the PSUM evict (no max subtraction; scores ~N(0,1)).
      O~.T[d,i] (+ softmax denominator row via the ones column) =
      v_aug[j,{d,1}].T @ expST[j,i], accumulated over j.
      Normalize: r = 1/s (reciprocal_approx_fast at evict); r broadcast along
      partitions via DRAM-bounce DMA for pairs 0-4 (latency hidden by deferred
      multiply emitters drained at jt==3 of a later pair), and via a K=1
      ones(x)r outer-product matmul into PSUM for the last pair (no DMA on the
      endgame critical path).
  C:  y[i,c] = sum_n OT[n,i].T @ wprojT[n,c] + b_proj (ones-row matmul).

Scheduling notes (what bought the time vs the naive ordering):
  - Input loads are need-ordered and issued from three engine rings (sync /
    scalar act / gpsimd) since each dma_start costs ~0.6us of issue time on
    its ring; PE warm-up dummies release the HAM clock throttle (1.2->2.4GHz)
    during the DMA wait.
  - Consecutive matmuls of one accumulation chain expose their LDWEIGHTS
    (~100ns each); everywhere two chains exist they are emitted interleaved
    (prologue A1 group pairs, in-B A1 pulls from two generators, phase C's two
    column chains share the per-gk stationary).  Phase-B steps are batched in
    twos (both S pairs, then both trailing O pairs) to halve the 64-row/
    128-row row-group-config transitions that also expose LDWEIGHTS.
  - The last pair runs ib=1 before ib=0 and phase C walks token blocks
    (4..7, 0..3) so the final normalize gates only work that is emitted last.
  - Output writeback is split into partition-quarter full-width-row chunks on
    two rings; a single big DMA drains on ~one queue.
"""

import sys

for p in ("/opt/trn_rl_repo", "/root/.axon_site/_ro/trn_rl_repo"):
    if p not in sys.path:
        sys.path.insert(0, p)

import numpy as np

import concourse.bacc as bacc
import concourse.mybir as mybir
import concourse.tile as tile

F32 = mybir.dt.float32
BF16 = mybir.dt.bfloat16

P = 128
NTOK = 1024
C = 768
H = 12
D = 64
SCALE = D ** -0.5
KT = C // P            # 6 contraction tiles over C
NT_Q = C // P          # 6 q feature tiles
JT = NTOK // P         # 8 key tiles
IT = NTOK // P         # 8 token tiles
VAW = H * (D + 1)      # 780: v_aug width


def build_attention(debug: bool = False):
    nc = bacc.Bacc("TRN2", target_bir_lowering=False, debug=False, num_devices=8)

    xT = nc.dram_tensor("xT", [C, NTOK], BF16, kind="ExternalInput")
    wqkvT = nc.dram_tensor("wqkvT", [C, 3 * C], BF16, kind="ExternalInput")
    wprojT = nc.dram_tensor("wprojT", [C, C], BF16, kind="ExternalInput")
    qkb = nc.dram_tensor("qkb", [P, NT_Q], F32, kind="ExternalInput")  # q_bias*SCALE, [p, nt]
    vb = nc.dram_tensor("vb", [1, C], BF16, kind="ExternalInput")
    bp = nc.dram_tensor("bp", [1, C], BF16, kind="ExternalInput")
    out = nc.dram_tensor("out", [NTOK, C], F32, kind="ExternalOutput")

    dbg = {}
    if debug:
        dbg["qkT"] = nc.dram_tensor("dbg_qkT", [2 * C, NTOK], BF16, kind="ExternalOutput")
        dbg["vaug"] = nc.dram_tensor("dbg_vaug", [IT * P, VAW], BF16, kind="ExternalOutput")
        dbg["ot"] = nc.dram_tensor("dbg_ot", [C, NTOK], BF16, kind="ExternalOutput")
        dbg["sden"] = nc.dram_tensor("dbg_sden", [H, NTOK], F32, kind="ExternalOutput")

    with tile.TileContext(nc) as tc:
        with tc.tile_pool(name="persist", bufs=1) as persist, \
             tc.tile_pool(name="dramscratch", bufs=1, space="DRAM") as dramp:
            rbounce = dramp.tile([H // 2, 2 * NTOK], BF16, tag="rbounce", name="rbounce")
            # persistent SBUF buffers
            qk_sb = [persist.tile([P, NTOK], BF16, tag=f"qk{t}", name=f"qk{t}") for t in range(12)]
            vaug_sb = [persist.tile([P, VAW], BF16, tag=f"vaug{t}", name=f"vaug{t}") for t in range(IT)]
            ot_sb = [persist.tile([P, NTOK], BF16, tag=f"ot{g}", name=f"ot{g}") for g in range(H // 2)]
            wp_sb = [persist.tile([P, C], BF16, tag=f"wp{t}", name=f"wp{t}") for t in range(KT)]
            ones_row = persist.tile([1, NTOK], BF16, tag="ones_row")
            ones_f32 = persist.tile([1, D], F32, tag="ones_f32")
            qkb_sb = persist.tile([P, NT_Q], F32, tag="qkb")
            vb_sb = persist.tile([1, C], BF16, tag="vb")
            bp_sb = persist.tile([1, C], BF16, tag="bp")

            nc.vector.memset(ones_row[:, :], 1.0)
            nc.vector.memset(ones_f32[:, :], 1.0)
            nc.sync.dma_start(out=qkb_sb[:, :], in_=qkb[:, :])

            # ---------------- Phases A+B fused ----------------
            # A1 emits qT/kT in single-bank [128,512] PSUM groups.  Pair 0's
            # q/k tiles and all of v (A2) are a prologue; the remaining ten A1
            # groups stream through a generator that phase B pulls from (2
            # matmuls per step), filling the PE slack under the exp stream.
            # PSUM budget: psA 2 banks + pss 4 banks + pso 2 banks = 8.
            with tc.tile_pool(name="work", bufs=1) as work, \
                 tc.tile_pool(name="psum", bufs=1, space="PSUM") as psum:
                pa = pb = pbs = work
                psA = psBs = psBo = psum
                xT_sb = [pa.tile([P, NTOK], BF16, tag=f"xT{t}", name=f"xTsb{t}") for t in range(KT)]
                wq_sb = [pa.tile([P, 3 * C], BF16, tag=f"wq{t}", name=f"wqsb{t}") for t in range(KT)]
                # PE warm-up: a few throwaway matmuls on ones_row release the
                # HAM clock throttle during the input-DMA wait.
                for w in range(4):
                    psd = psA.tile([P, 512], F32, tag="psA1", name=f"psd{w}", bufs=2)
                    nc.tensor.matmul(psd[:, :], ones_row[:, 0:P], ones_row[:, 0:512],
                                     start=True, stop=True)
                # Need-ordered input loads.  Descriptor issue costs ~0.6us per
                # dma_start on the issuing engine, so spread across four engine
                # queues: first A1 group's deps (xT ib=0 halves + pair-0 q/k
                # weight slices) go first on dedicated queues.
                for t in range(KT):
                    nc.sync.dma_start(out=xT_sb[t][:, 0:512], in_=xT[t * P:(t + 1) * P, 0:512])
                for t in range(KT):
                    nc.sync.dma_start(out=xT_sb[t][:, 512:NTOK], in_=xT[t * P:(t + 1) * P, 512:NTOK])
                for t in range(KT):
                    nc.scalar.dma_start(out=wq_sb[t][:, 0:P], in_=wqkvT[t * P:(t + 1) * P, 0:P])
                for t in range(KT):
                    nc.scalar.dma_start(out=wq_sb[t][:, C:C + P], in_=wqkvT[t * P:(t + 1) * P, C:C + P])
                nc.gpsimd.dma_start(out=vb_sb[:, :], in_=vb[:, :])
                nc.gpsimd.dma_start(out=bp_sb[:, :], in_=bp[:, :])
                for t in range(KT):
                    nc.gpsimd.dma_start(out=wq_sb[t][:, P:2 * P], in_=wqkvT[t * P:(t + 1) * P, P:2 * P])
                    nc.gpsimd.dma_start(out=wq_sb[t][:, C + P:C + 2 * P], in_=wqkvT[t * P:(t + 1) * P, C + P:C + 2 * P])
                for t in range(KT):
                    nc.gpsimd.dma_start(out=wq_sb[t][:, 2 * C:3 * C], in_=wqkvT[t * P:(t + 1) * P, 2 * C:3 * C])
                for t in range(KT):
                    nc.gpsimd.dma_start(out=wq_sb[t][:, 2 * P:C], in_=wqkvT[t * P:(t + 1) * P, 2 * P:C])
                    nc.gpsimd.dma_start(out=wq_sb[t][:, C + 2 * P:2 * C], in_=wqkvT[t * P:(t + 1) * P, C + 2 * P:2 * C])

                def a1_group1(nt, ib):
                    ps = psA.tile([P, 512], F32, tag="psA1", name="psa1", bufs=2)
                    for kt in range(KT):
                        nc.tensor.matmul(
                            ps[:, :],
                            wq_sb[kt][:, nt * P:(nt + 1) * P],
                            xT_sb[kt][:, ib * 512:(ib + 1) * 512],
                            start=(kt == 0), stop=(kt == KT - 1))
                        yield
                    if nt < NT_Q:
                        nc.vector.tensor_scalar(
                            out=qk_sb[nt][:, ib * 512:(ib + 1) * 512], in0=ps[:, :],
                            scalar1=float(SCALE), scalar2=qkb_sb[:, nt:nt + 1],
                            op0=mybir.AluOpType.mult, op1=mybir.AluOpType.add)
                    else:
                        nc.vector.tensor_copy(
                            qk_sb[nt][:, ib * 512:(ib + 1) * 512], ps[:, :])
                    yield

                def a1_groups_gen(nts):
                    for nt in nts:
                        for ib in range(2):
                            yield from a1_group1(nt, ib)

                # prologue: q/k tiles for pairs 0 and 1; v streams through
                # pair 0's steps, the remaining A1 groups through pairs 1-4.
                # Emit the two in-flight groups' matmuls interleaved so each
                # LDWEIGHTS hides under the other chain's streaming matmul.
                _SENT = object()
                for nt_a, nt_b in ((0, 6), (1, 7), (2, 8)):
                    for ib in range(2):
                        gens = [a1_group1(nt_a, ib), a1_group1(nt_b, ib)]
                        while gens:
                            gens = [g for g in gens if next(g, _SENT) is not _SENT]

                def a2_gen():
                    for tt in range(IT):
                        psv = [psA.tile([P, 384], F32, tag="psA1", name=f"psv{v}", bufs=2) for v in range(2)]
                        for kt in range(KT):
                            for vbk in range(2):
                                nc.tensor.matmul(
                                    psv[vbk][:, :],
                                    xT_sb[kt][:, tt * P:(tt + 1) * P],
                                    wq_sb[kt][:, 2 * C + vbk * 384: 2 * C + (vbk + 1) * 384],
                                    start=(kt == 0), stop=False)
                                yield
                        for vbk in range(2):
                            nc.tensor.matmul(
                                psv[vbk][:, :],
                                ones_row[:, tt * P:(tt + 1) * P],
                                vb_sb[:, vbk * 384:(vbk + 1) * 384],
                                start=False, stop=True)
                            dst = vaug_sb[tt][:, :].rearrange("p (h e) -> p h e", e=D + 1)
                            nc.vector.tensor_copy(
                                dst[:, 6 * vbk:6 * (vbk + 1), 0:D],
                                psv[vbk][:, :].rearrange("p (h d) -> p h d", d=D))
                            yield
                        ones_cols = vaug_sb[tt][:, :].rearrange("p (h e) -> p h e", e=D + 1)[:, :, D:D + 1]
                        nc.vector.memset(ones_cols, 1.0)
                        yield

                for t in range(KT):
                    nc.gpsimd.dma_start(out=wp_sb[t][:, :], in_=wprojT[t * P:(t + 1) * P, :])

                a2_work = a2_gen()

                def interleave_gens(g1, g2):
                    # one matmul from each chain per yield: consecutive PE
                    # instructions come from different chains so LDWEIGHTS
                    # pulls ahead instead of serializing.
                    gens = [g1, g2]
                    while gens:
                        for gg in list(gens):
                            if next(gg, _SENT) is _SENT:
                                gens.remove(gg)
                            else:
                                yield

                a1_rest = interleave_gens(
                    a1_groups_gen((3, 4, 5)), a1_groups_gen((9, 10, 11)))

                def emit_o_mms(gp, pso, exp_, ib, jt):
                    for e in range(2):
                        nc.tensor.matmul(
                            pso[e][:, :],
                            vaug_sb[jt][:, (2 * gp + e) * (D + 1):(2 * gp + e + 1) * (D + 1)],
                            exp_[(ib, jt)][:, e * 512:(e + 1) * 512],
                            start=(jt == 0), stop=(jt == JT - 1))

                def evict_o_half(gp, pso, ib, sdr):
                    # O rows to SBUF; softmax denominators via an SBUF row
                    # bounce (copy from PSUM partition 64), then reciprocal.
                    for e in range(2):
                        nc.vector.tensor_copy(
                            ot_sb[gp][e * D:(e + 1) * D, ib * 512:(ib + 1) * 512],
                            pso[e][0:D, :])
                        nc.vector.tensor_copy(
                            sdr[0:1, (2 + e) * NTOK + ib * 512:(2 + e) * NTOK + (ib + 1) * 512],
                            pso[e][D:D + 1, :])
                        nc.vector.reciprocal_approx_fast(
                            sdr[0:1, e * NTOK + ib * 512:e * NTOK + (ib + 1) * 512],
                            sdr[0:1, (2 + e) * NTOK + ib * 512:(2 + e) * NTOK + (ib + 1) * 512])

                def finish_norm_half(gp, sdr, ib):
                    # deferred: broadcast r along partitions via a K=1 outer
                    # product on the PE (ones x r), multiply from PSUM.  No
                    # DRAM bounce, no DMA on the critical path.
                    if debug:
                        for e in range(2):
                            nc.sync.dma_start(
                                out=dbg["sden"][2 * gp + e:2 * gp + e + 1, ib * 512:(ib + 1) * 512],
                                in_=sdr[0:1, e * NTOK + ib * 512:e * NTOK + (ib + 1) * 512])

                    def fin():
                        rps = psBs.tile([P, 512], F32, tag="pss", bufs=2, name="rps")
                        for e in range(2):
                            nc.tensor.matmul(
                                rps[e * D:(e + 1) * D, :],
                                ones_f32[0:1, 0:D],
                                sdr[0:1, e * NTOK + ib * 512:e * NTOK + (ib + 1) * 512],
                                start=True, stop=True)
                        nc.vector.tensor_mul(
                            ot_sb[gp][:, ib * 512:(ib + 1) * 512],
                            ot_sb[gp][:, ib * 512:(ib + 1) * 512], rps[:, :])
                    return fin

                def finish_norm_dma(gp, sdr):
                    # pairs 0-4: broadcast r via DRAM bounce on idle DMA
                    # engines; only the final multiply is deferred.
                    if debug:
                        for e in range(2):
                            nc.sync.dma_start(
                                out=dbg["sden"][2 * gp + e:2 * gp + e + 1, :],
                                in_=sdr[0:1, e * NTOK:(e + 1) * NTOK])
                    rbF = pbs.tile([1, 2 * NTOK], BF16, tag="rbf", bufs=2)
                    nc.vector.tensor_copy(rbF[0:1, :], sdr[0:1, 0:2 * NTOK])
                    nc.sync.dma_start(out=rbounce[gp:gp + 1, :], in_=rbF[0:1, :])
                    rp = pbs.tile([P, NTOK], BF16, tag="rpair", bufs=3)
                    for e in range(2):
                        nc.sync.dma_start(
                            out=rp[e * D:(e + 1) * D, :],
                            in_=rbounce[gp:gp + 1,
                                        e * NTOK:(e + 1) * NTOK].partition_broadcast(D))

                    def fin():
                        nc.vector.tensor_mul(ot_sb[gp][:, :], ot_sb[gp][:, :], rp[:, :])
                    return fin

                # Pair g's S/exp stream; pair g-1's O matmuls trail inside it
                # (sequential over ib so the O accumulators take 2 PSUM banks).
                prev = None  # (g, ex, stg2) for cross-pair pairs 0..3
                A1_PULLS = {1: 2, 2: 2, 3: 2}
                SELF_PAIRS = (4, 5)  # O runs same-pair; normalized per-ib in-pair
                pending_fin = []     # deferred normalize broadcast+multiply emitters
                for g in range(H // 2):
                    self_o = g in SELF_PAIRS
                    ex = {}
                    sdr = pbs.tile([1, 4 * NTOK], F32, tag="sdrec", name="sdrec", bufs=2)
                    ibs = (1, 0) if g == H // 2 - 1 else (0, 1)
                    for ib in ibs:
                        pso_prev = None
                        if prev is not None:
                            pso_prev = [psBo.tile([D + 1, 512], F32, tag=f"pso{e}", name=f"pso{e}")
                                        for e in range(2)]
                        pso_self = None
                        if self_o:
                            if g == 4:
                                pso_self = [psA.tile([D + 1, 512], F32, tag="psA1", name=f"psoL{e}", bufs=2)
                                            for e in range(2)]
                            else:
                                pso_self = [psBo.tile([D + 1, 512], F32, tag=f"pso{e}", name=f"psoM{e}")
                                            for e in range(2)]
                        # steps batched in twos: both S pairs, then both
                        # trailing O pairs -- halves the row-group-config
                        # transitions that expose LDWEIGHTS on the PE.
                        for jt2 in range(0, JT, 2):
                            for jt in (jt2, jt2 + 1):
                                if jt == 3:
                                    for f in pending_fin:
                                        f()
                                    pending_fin = []
                                if g == 0:
                                    for _ in range(8):
                                        next(a2_work, None)
                                else:
                                    for _ in range(A1_PULLS.get(g, 0)):
                                        next(a1_rest, None)
                                pss = psBs.tile([P, NTOK], F32, tag="pss", bufs=2)
                                for e in range(2):
                                    nc.tensor.matmul(
                                        pss[:, e * 512:(e + 1) * 512],
                                        qk_sb[NT_Q + g][e * D:(e + 1) * D, jt * P:(jt + 1) * P],
                                        qk_sb[g][e * D:(e + 1) * D, ib * 512:(ib + 1) * 512],
                                        start=True, stop=True, tile_position=(e * D, 0))
                                et = pb.tile([P, NTOK], BF16, tag="expst", bufs=24)
                                nc.scalar.activation(et[:, :], pss[:, :], mybir.ActivationFunctionType.Exp)
                                ex[(ib, jt)] = et
                            for jt in (jt2 - 2, jt2 - 1):
                                if jt < 0:
                                    continue
                                if pso_prev is not None:
                                    emit_o_mms(prev[0], pso_prev, prev[1], ib, jt)
                                if pso_self is not None:
                                    emit_o_mms(g, pso_self, ex, ib, jt)
                        for jt in (JT - 2, JT - 1):
                            if pso_prev is not None:
                                emit_o_mms(prev[0], pso_prev, prev[1], ib, jt)
                            if pso_self is not None:
                                emit_o_mms(g, pso_self, ex, ib, jt)
                        if pso_prev is not None:
                            evict_o_half(prev[0], pso_prev, ib, prev[2])
                        if pso_self is not None:
                            evict_o_half(g, pso_self, ib, sdr)
                            if g == H // 2 - 1:
                                pending_fin.append(finish_norm_half(g, sdr, ib))
                    if self_o and g == 4:
                        pending_fin.append(finish_norm_dma(g, sdr))
                    if prev is not None:
                        pending_fin.append(finish_norm_dma(prev[0], prev[2]))
                    prev = None if self_o else (g, ex, sdr)
                for _ in a1_rest:
                    pass

                # ---------------- Phase C: output projection ----------------
                # Shares PSUM slots with tag "pss" -- no pool barrier, so C
                # starts as soon as the last pair is normalized.
                psy_tags = ((psBs, "pss"), (psBs, "pss"), (psBo, "pso0"),
                            (psBo, "pso1"), (psA, "psA1"), (psA, "psA1"))
                ci = 0
                for it in (4, 5, 6, 7, 0, 1, 2, 3):
                    if it == 0:
                        # last pair's ib=0 normalize: emit after the second-
                        # token-half chains so its reciprocal has landed.
                        for f in pending_fin:
                            f()
                        pending_fin = []
                    o_tile = pb.tile([P, C], F32, tag="yout", bufs=3)
                    psys = []
                    for cb, cw in ((0, 512), (512, 256)):
                        cpool, ctag = psy_tags[ci % 6]
                        ci += 1
                        psys.append((cpool.tile([P, 512], F32, tag=ctag, name="psy",
                                                 bufs=2 if ctag in ("pss", "psA1") else 1), cb, cw))
                    # both column chains share the per-gk stationary: emit them
                    # adjacently so LDWEIGHTS loads once per gk and hides under
                    # the other chain's streaming matmul.
                    for gk in range(KT):
                        for psy, cb, cw in psys:
                            nc.tensor.matmul(
                                psy[:, 0:cw],
                                ot_sb[gk][:, it * P:(it + 1) * P],
                                wp_sb[gk][:, cb:cb + cw],
                                start=(gk == 0), stop=False)
                    for psy, cb, cw in psys:
                        nc.tensor.matmul(
                            psy[:, 0:cw],
                            ones_row[:, it * P:(it + 1) * P],
                            bp_sb[:, cb:cb + cw],
                            start=False, stop=True)
                    # evict on Scalar (idle after phase B) for the wide
                    # half, Vector for the narrow one -- C never waits.
                    for psy, cb, cw in psys:
                        if cb == 0:
                            nc.scalar.activation(
                                o_tile[:, 0:512], psy[:, 0:512],
                                mybir.ActivationFunctionType.Copy)
                        else:
                            nc.vector.tensor_copy(o_tile[:, cb:cb + cw], psy[:, 0:cw])
                    # writeback in partition-quarter chunks with full-width
                    # rows (3KB contiguous DRAM descriptors), two rings -- a
                    # single DMA otherwise drains on ~one queue.
                    for q, ring in ((0, nc.sync), (1, nc.gpsimd), (2, nc.sync), (3, nc.gpsimd)):
                        ring.dma_start(
                            out=out[it * P + q * 32:it * P + (q + 1) * 32, :],
                            in_=o_tile[q * 32:(q + 1) * 32, :])

            if debug:
                for t in range(12):
                    nc.sync.dma_start(out=dbg["qkT"][t * P:(t + 1) * P, :], in_=qk_sb[t][:, :])
                for t in range(IT):
                    nc.sync.dma_start(out=dbg["vaug"][t * P:(t + 1) * P, :], in_=vaug_sb[t][:, :])
                for g in range(H // 2):
                    nc.sync.dma_start(out=dbg["ot"][g * P:(g + 1) * P, :], in_=ot_sb[g][:, :])

    nc.compile()
    return nc


_CACHED_NC = {}


def get_compiled(debug: bool = False):
    if debug not in _CACHED_NC:
        _CACHED_NC[debug] = build_attention(debug)
    return _CACHED_NC[debug]


def prep_inputs(x, w_qkv, q_bias, v_bias, w_proj, b_proj):
    x = np.asarray(x, np.float32)
    B = x.shape[0]
    import ml_dtypes
    bf16 = ml_dtypes.bfloat16
    xT = np.ascontiguousarray(x.transpose(0, 2, 1)).astype(bf16)
    wqkvT = np.ascontiguousarray(np.asarray(w_qkv, np.float32).T).astype(bf16)
    wprojT = np.ascontiguousarray(np.asarray(w_proj, np.float32).T).astype(bf16)
    qkb = np.ascontiguousarray((np.asarray(q_bias, np.float32) * SCALE).reshape(NT_Q, P).T)
    vbr = np.ascontiguousarray(np.asarray(v_bias, np.float32).reshape(1, C)).astype(bf16)
    bpr = np.ascontiguousarray(np.asarray(b_proj, np.float32).reshape(1, C)).astype(bf16)
    return [
        {"xT": xT[i], "wqkvT": wqkvT, "wprojT": wprojT, "qkb": qkb, "vb": vbr, "bp": bpr}
        for i in range(B)
    ]




def kernel(**inputs):
    """Harness entrypoint: full inputs in, full output out (8-way data parallel)."""
    from concourse.bass_utils import run_bass_kernel_spmd

    nc = get_compiled(False)
    in_maps = prep_inputs(
        inputs["x"], inputs["w_qkv"], inputs["q_bias"], inputs["v_bias"],
        inputs["w_proj"], inputs["b_proj"])
    res = run_bass_kernel_spmd(nc, in_maps, core_ids=list(range(8)))
    return np.stack([res.results[i]["out"] for i in range(len(in_maps))])



# revision 30
# speedup vs baseline: 1.0097x; 1.0097x over previous
"""Multi-head attention (B=8, N=1024, C=768, H=12, D=64) on 8 TRN2 NeuronCores.

Sharding: pure data parallel — one batch element per core, no collectives.

Per-core dataflow (all matmuls laid out so no on-device transposes are needed):
  A1: qT/kT feature-major:   qkvT[n,i] = sum_c wqkvT[c,n] * xT[c,i]
      (w stationary, xT moving).  q rows get *SCALE and +q_bias*SCALE on evict.
  A2: v token-major:         v[i,f] = sum_c xT[c,i] * wqkvT[c,f] + v_bias[f]
      (xT stationary, w moving; bias via ones-row matmul).  Evicted bf16 into
      v_aug layout [tok, h*65+d] with a ones column per head at d=64.
  B:  per head pair g: S.T[j,i] = kT_h[d,j].T @ qT_h[d,i]  (K=64, heads 2g /
      2g+1 in partition rows 0:64 / 64:128 -> concurrent row-group matmuls).
      exp on the PSUM evict (no max subtraction; scores ~N(0,1)).
      O~.T[d,i] (+ softmax denominator row via the ones column) =
      v_aug[j,{d,1}].T @ expST[j,i], accumulated over j.
      Normalize: r = 1/s (reciprocal_approx_fast at evict); r broadcast along
      partitions via DRAM-bounce DMA for pairs 0-4 (latency hidden by deferred
      multiply emitters drained at jt==3 of a later pair), and via a K=1
      ones(x)r outer-product matmul into PSUM for the last pair (no DMA on the
      endgame critical path).
  C:  y[i,c] = sum_n OT[n,i].T @ wprojT[n,c] + b_proj (ones-row matmul).

Scheduling notes (what bought the time vs the naive ordering):
  - Input loads are need-ordered and issued from three engine rings (sync /
    scalar act / gpsimd) since each dma_start costs ~0.6us of issue time on
    its ring; PE warm-up dummies release the HAM clock throttle (1.2->2.4GHz)
    during the DMA wait.
  - Consecutive matmuls of one accumulation chain expose their LDWEIGHTS
    (~100ns each); everywhere two chains exist they are emitted interleaved
    (prologue A1 group pairs, in-B A1 pulls from two generators, phase C's two
    column chains share the per-gk stationary).  Phase-B steps are batched in
    twos (both S pairs, then both trailing O pairs) to halve the 64-row/
    128-row row-group-config transitions that also expose LDWEIGHTS.
  - The last pair runs ib=1 before ib=0 and phase C walks token blocks
    (4..7, 0..3) so the final normalize gates only work that is emitted last.
  - Output writeback is split into partition-quarter full-width-row chunks on
    two rings; a single big DMA drains on ~one queue.
"""

import sys

for p in ("/opt/trn_rl_repo", "/root/.axon_site/_ro/trn_rl_repo"):
    if p not in sys.path:
        sys.path.insert(0, p)

import numpy as np

import concourse.bacc as bacc
import concourse.mybir as mybir
import concourse.tile as tile

F32 = mybir.dt.float32
BF16 = mybir.dt.bfloat16

P = 128
NTOK = 1024
C = 768
H = 12
D = 64
SCALE = D ** -0.5
KT = C // P            # 6 contraction tiles over C
NT_Q = C // P          # 6 q feature tiles
JT = NTOK // P         # 8 key tiles
IT = NTOK // P         # 8 token tiles
VAW = H * (D + 1)      # 780: v_aug width


def build_attention(debug: bool = False):
    nc = bacc.Bacc("TRN2", target_bir_lowering=False, debug=False, num_devices=8)

    xT = nc.dram_tensor("xT", [C, NTOK], BF16, kind="ExternalInput")
    wqkvT = nc.dram_tensor("wqkvT", [C, 3 * C], BF16, kind="ExternalInput")
    wprojT = nc.dram_tensor("wprojT", [C, C], BF16, kind="ExternalInput")
    qkb = nc.dram_tensor("qkb", [P, NT_Q], F32, kind="ExternalInput")  # q_bias*SCALE, [p, nt]
    vb = nc.dram_tensor("vb", [1, C], BF16, kind="ExternalInput")
    bp = nc.dram_tensor("bp", [1, C], BF16, kind="ExternalInput")
    out = nc.dram_tensor("out", [NTOK, C], F32, kind="ExternalOutput")

    dbg = {}
    if debug:
        dbg["qkT"] = nc.dram_tensor("dbg_qkT", [2 * C, NTOK], BF16, kind="ExternalOutput")
        dbg["vaug"] = nc.dram_tensor("dbg_vaug", [IT * P, VAW], BF16, kind="ExternalOutput")
        dbg["ot"] = nc.dram_tensor("dbg_ot", [C, NTOK], BF16, kind="ExternalOutput")
        dbg["sden"] = nc.dram_tensor("dbg_sden", [H, NTOK], F32, kind="ExternalOutput")

    with tile.TileContext(nc) as tc:
        with tc.tile_pool(name="persist", bufs=1) as persist, \
             tc.tile_pool(name="dramscratch", bufs=1, space="DRAM") as dramp:
            rbounce = dramp.tile([H // 2, 2 * NTOK], BF16, tag="rbounce", name="rbounce")
            # persistent SBUF buffers
            qk_sb = [persist.tile([P, NTOK], BF16, tag=f"qk{t}", name=f"qk{t}") for t in range(12)]
            vaug_sb = [persist.tile([P, VAW], BF16, tag=f"vaug{t}", name=f"vaug{t}") for t in range(IT)]
            ot_sb = [persist.tile([P, NTOK], BF16, tag=f"ot{g}", name=f"ot{g}") for g in range(H // 2)]
            wp_sb = [persist.tile([P, C], BF16, tag=f"wp{t}", name=f"wp{t}") for t in range(KT)]
            ones_row = persist.tile([1, NTOK], BF16, tag="ones_row")
            ones_f32 = persist.tile([1, D], F32, tag="ones_f32")
            qkb_sb = persist.tile([P, NT_Q], F32, tag="qkb")
            vb_sb = persist.tile([1, C], BF16, tag="vb")
            bp_sb = persist.tile([1, C], BF16, tag="bp")

            nc.vector.memset(ones_row[:, :], 1.0)
            nc.vector.memset(ones_f32[:, :], 1.0)
            nc.sync.dma_start(out=qkb_sb[:, :], in_=qkb[:, :])

            # ---------------- Phases A+B fused ----------------
            # A1 emits qT/kT in single-bank [128,512] PSUM groups.  Pair 0's
            # q/k tiles and all of v (A2) are a prologue; the remaining ten A1
            # groups stream through a generator that phase B pulls from (2
            # matmuls per step), filling the PE slack under the exp stream.
            # PSUM budget: psA 2 banks + pss 4 banks + pso 2 banks = 8.
            with tc.tile_pool(name="work", bufs=1) as work, \
                 tc.tile_pool(name="psum", bufs=1, space="PSUM") as psum:
                pa = pb = pbs = work
                psA = psBs = psBo = psum
                xT_sb = [pa.tile([P, NTOK], BF16, tag=f"xT{t}", name=f"xTsb{t}") for t in range(KT)]
                wq_sb = [pa.tile([P, 3 * C], BF16, tag=f"wq{t}", name=f"wqsb{t}") for t in range(KT)]
                # PE warm-up: a few throwaway matmuls on ones_row release the
                # HAM clock throttle during the input-DMA wait.
                for w in range(4):
                    psd = psA.tile([P, 512], F32, tag="psA1", name=f"psd{w}", bufs=2)
                    nc.tensor.matmul(psd[:, :], ones_row[:, 0:P], ones_row[:, 0:512],
                                     start=True, stop=True)
                # Need-ordered input loads.  Descriptor issue costs ~0.6us per
                # dma_start on the issuing engine, so spread across four engine
                # queues: first A1 group's deps (xT ib=0 halves + pair-0 q/k
                # weight slices) go first on dedicated queues.
                for t in range(KT):
                    nc.sync.dma_start(out=xT_sb[t][:, 0:512], in_=xT[t * P:(t + 1) * P, 0:512])
                for t in range(KT):
                    nc.sync.dma_start(out=xT_sb[t][:, 512:NTOK], in_=xT[t * P:(t + 1) * P, 512:NTOK])
                for t in range(KT):
                    nc.scalar.dma_start(out=wq_sb[t][:, 0:P], in_=wqkvT[t * P:(t + 1) * P, 0:P])
                for t in range(KT):
                    nc.scalar.dma_start(out=wq_sb[t][:, C:C + P], in_=wqkvT[t * P:(t + 1) * P, C:C + P])
                nc.gpsimd.dma_start(out=vb_sb[:, :], in_=vb[:, :])
                nc.gpsimd.dma_start(out=bp_sb[:, :], in_=bp[:, :])
                for t in range(KT):
                    nc.gpsimd.dma_start(out=wq_sb[t][:, P:2 * P], in_=wqkvT[t * P:(t + 1) * P, P:2 * P])
                    nc.gpsimd.dma_start(out=wq_sb[t][:, C + P:C + 2 * P], in_=wqkvT[t * P:(t + 1) * P, C + P:C + 2 * P])
                for t in range(KT):
                    nc.gpsimd.dma_start(out=wq_sb[t][:, 2 * C:3 * C], in_=wqkvT[t * P:(t + 1) * P, 2 * C:3 * C])
                for t in range(KT):
                    nc.gpsimd.dma_start(out=wq_sb[t][:, 2 * P:C], in_=wqkvT[t * P:(t + 1) * P, 2 * P:C])
                    nc.gpsimd.dma_start(out=wq_sb[t][:, C + 2 * P:2 * C], in_=wqkvT[t * P:(t + 1) * P, C + 2 * P:2 * C])

                def a1_group1(nt, ib):
                    ps = psA.tile([P, 512], F32, tag="psA1", name="psa1", bufs=2)
                    for kt in range(KT):
                        nc.tensor.matmul(
                            ps[:, :],
                            wq_sb[kt][:, nt * P:(nt + 1) * P],
                            xT_sb[kt][:, ib * 512:(ib + 1) * 512],
                            start=(kt == 0), stop=(kt == KT - 1))
                        yield
                    if nt < NT_Q:
                        nc.vector.tensor_scalar(
                            out=qk_sb[nt][:, ib * 512:(ib + 1) * 512], in0=ps[:, :],
                            scalar1=float(SCALE), scalar2=qkb_sb[:, nt:nt + 1],
                            op0=mybir.AluOpType.mult, op1=mybir.AluOpType.add)
                    else:
                        nc.vector.tensor_copy(
                            qk_sb[nt][:, ib * 512:(ib + 1) * 512], ps[:, :])
                    yield

                def a1_groups_gen(nts):
                    for nt in nts:
                        for ib in range(2):
                            yield from a1_group1(nt, ib)

                # prologue: q/k tiles for pairs 0 and 1; v streams through
                # pair 0's steps, the remaining A1 groups through pairs 1-4.
                # Emit the two in-flight groups' matmuls interleaved so each
                # LDWEIGHTS hides under the other chain's streaming matmul.
                _SENT = object()
                for nt_a, nt_b in ((0, 6), (1, 7), (2, 8)):
                    for ib in range(2):
                        gens = [a1_group1(nt_a, ib), a1_group1(nt_b, ib)]
                        while gens:
                            gens = [g for g in gens if next(g, _SENT) is not _SENT]

                def a2_gen():
                    for tt in range(IT):
                        psv = [psA.tile([P, 384], F32, tag="psA1", name=f"psv{v}", bufs=2) for v in range(2)]
                        for kt in range(KT):
                            for vbk in range(2):
                                nc.tensor.matmul(
                                    psv[vbk][:, :],
                                    xT_sb[kt][:, tt * P:(tt + 1) * P],
                                    wq_sb[kt][:, 2 * C + vbk * 384: 2 * C + (vbk + 1) * 384],
                                    start=(kt == 0), stop=False)
                                yield
                        for vbk in range(2):
                            nc.tensor.matmul(
                                psv[vbk][:, :],
                                ones_row[:, tt * P:(tt + 1) * P],
                                vb_sb[:, vbk * 384:(vbk + 1) * 384],
                                start=False, stop=True)
                            dst = vaug_sb[tt][:, :].rearrange("p (h e) -> p h e", e=D + 1)
                            nc.vector.tensor_copy(
                                dst[:, 6 * vbk:6 * (vbk + 1), 0:D],
                                psv[vbk][:, :].rearrange("p (h d) -> p h d", d=D))
                            yield
                        ones_cols = vaug_sb[tt][:, :].rearrange("p (h e) -> p h e", e=D + 1)[:, :, D:D + 1]
                        nc.vector.memset(ones_cols, 1.0)
                        yield

                for t in range(KT):
                    nc.gpsimd.dma_start(out=wp_sb[t][:, :], in_=wprojT[t * P:(t + 1) * P, :])

                a2_work = a2_gen()

                def interleave_gens(g1, g2):
                    # one matmul from each chain per yield: consecutive PE
                    # instructions come from different chains so LDWEIGHTS
                    # pulls ahead instead of serializing.
                    gens = [g1, g2]
                    while gens:
                        for gg in list(gens):
                            if next(gg, _SENT) is _SENT:
                                gens.remove(gg)
                            else:
                                yield

                a1_rest = interleave_gens(
                    a1_groups_gen((3, 4, 5)), a1_groups_gen((9, 10, 11)))

                def emit_o_mms(gp, pso, exp_, ib, jt):
                    for e in range(2):
                        nc.tensor.matmul(
                            pso[e][:, :],
                            vaug_sb[jt][:, (2 * gp + e) * (D + 1):(2 * gp + e + 1) * (D + 1)],
                            exp_[(ib, jt)][:, e * 512:(e + 1) * 512],
                            start=(jt == 0), stop=(jt == JT - 1))

                def evict_o_half(gp, pso, ib, sdr):
                    # O rows to SBUF; softmax denominators via an SBUF row
                    # bounce (copy from PSUM partition 64), then reciprocal.
                    for e in range(2):
                        nc.vector.tensor_copy(
                            ot_sb[gp][e * D:(e + 1) * D, ib * 512:(ib + 1) * 512],
                            pso[e][0:D, :])
                        nc.vector.tensor_copy(
                            sdr[0:1, (2 + e) * NTOK + ib * 512:(2 + e) * NTOK + (ib + 1) * 512],
                            pso[e][D:D + 1, :])
                        nc.vector.reciprocal_approx_fast(
                            sdr[0:1, e * NTOK + ib * 512:e * NTOK + (ib + 1) * 512],
                            sdr[0:1, (2 + e) * NTOK + ib * 512:(2 + e) * NTOK + (ib + 1) * 512])

                def finish_norm_half(gp, sdr, ib):
                    # deferred: broadcast r along partitions via a K=1 outer
                    # product on the PE (ones x r), multiply from PSUM.  No
                    # DRAM bounce, no DMA on the critical path.
                    if debug:
                        for e in range(2):
                            nc.sync.dma_start(
                                out=dbg["sden"][2 * gp + e:2 * gp + e + 1, ib * 512:(ib + 1) * 512],
                                in_=sdr[0:1, e * NTOK + ib * 512:e * NTOK + (ib + 1) * 512])

                    def fin():
                        rps = psBs.tile([P, 512], F32, tag="pss", bufs=2, name="rps")
                        for e in range(2):
                            nc.tensor.matmul(
                                rps[e * D:(e + 1) * D, :],
                                ones_f32[0:1, 0:D],
                                sdr[0:1, e * NTOK + ib * 512:e * NTOK + (ib + 1) * 512],
                                start=True, stop=True)
                        nc.vector.tensor_mul(
                            ot_sb[gp][:, ib * 512:(ib + 1) * 512],
                            ot_sb[gp][:, ib * 512:(ib + 1) * 512], rps[:, :])
                    return fin

                def finish_norm_dma(gp, sdr):
                    # pairs 0-4: broadcast r via DRAM bounce on idle DMA
                    # engines; only the final multiply is deferred.
                    if debug:
                        for e in range(2):
                            nc.sync.dma_start(
                                out=dbg["sden"][2 * gp + e:2 * gp + e + 1, :],
                                in_=sdr[0:1, e * NTOK:(e + 1) * NTOK])
                    rbF = pbs.tile([1, 2 * NTOK], BF16, tag="rbf", bufs=2)
                    nc.vector.tensor_copy(rbF[0:1, :], sdr[0:1, 0:2 * NTOK])
                    nc.sync.dma_start(out=rbounce[gp:gp + 1, :], in_=rbF[0:1, :])
                    rp = pbs.tile([P, NTOK], BF16, tag="rpair", bufs=3)
                    for e in range(2):
                        nc.sync.dma_start(
                            out=rp[e * D:(e + 1) * D, :],
                            in_=rbounce[gp:gp + 1,
                                        e * NTOK:(e + 1) * NTOK].partition_broadcast(D))

                    def fin():
                        nc.vector.tensor_mul(ot_sb[gp][:, :], ot_sb[gp][:, :], rp[:, :])
                    return fin

                # Pair g's S/exp stream; pair g-1's O matmuls trail inside it
                # (sequential over ib so the O accumulators take 2 PSUM banks).
                prev = None  # (g, ex, stg2) for cross-pair pairs 0..3
                A1_PULLS = {1: 2, 2: 2, 3: 2}
                SELF_PAIRS = (4, 5)  # O runs same-pair; normalized per-ib in-pair
                pending_fin = []     # deferred normalize broadcast+multiply emitters
                for g in range(H // 2):
                    self_o = g in SELF_PAIRS
                    ex = {}
                    sdr = pbs.tile([1, 4 * NTOK], F32, tag="sdrec", name="sdrec", bufs=2)
                    ibs = (1, 0) if g == H // 2 - 1 else (0, 1)
                    for ib in ibs:
                        pso_prev = None
                        if prev is not None:
                            pso_prev = [psBo.tile([D + 1, 512], F32, tag=f"pso{e}", name=f"pso{e}")
                                        for e in range(2)]
                        pso_self = None
                        if self_o:
                            if g == 4:
                                pso_self = [psA.tile([D + 1, 512], F32, tag="psA1", name=f"psoL{e}", bufs=2)
                                            for e in range(2)]
                            else:
                                pso_self = [psBo.tile([D + 1, 512], F32, tag=f"pso{e}", name=f"psoM{e}")
                                            for e in range(2)]
                        # steps batched in twos: both S pairs, then both
                        # trailing O pairs -- halves the row-group-config
                        # transitions that expose LDWEIGHTS on the PE.
                        for jt2 in range(0, JT, 2):
                            for jt in (jt2, jt2 + 1):
                                if jt == 3:
                                    for f in pending_fin:
                                        f()
                                    pending_fin = []
                                if g == 0:
                                    for _ in range(8):
                                        next(a2_work, None)
                                else:
                                    for _ in range(A1_PULLS.get(g, 0)):
                                        next(a1_rest, None)
                                pss = psBs.tile([P, NTOK], F32, tag="pss", bufs=2)
                                for e in range(2):
                                    nc.tensor.matmul(
                                        pss[:, e * 512:(e + 1) * 512],
                                        qk_sb[NT_Q + g][e * D:(e + 1) * D, jt * P:(jt + 1) * P],
                                        qk_sb[g][e * D:(e + 1) * D, ib * 512:(ib + 1) * 512],
                                        start=True, stop=True, tile_position=(e * D, 0))
                                et = pb.tile([P, NTOK], BF16, tag="expst", bufs=24)
                                nc.scalar.activation(et[:, :], pss[:, :], mybir.ActivationFunctionType.Exp)
                                ex[(ib, jt)] = et
                            for jt in (jt2 - 2, jt2 - 1):
                                if jt < 0:
                                    continue
                                if pso_prev is not None:
                                    emit_o_mms(prev[0], pso_prev, prev[1], ib, jt)
                                if pso_self is not None:
                                    emit_o_mms(g, pso_self, ex, ib, jt)
                        for jt in (JT - 2, JT - 1):
                            if pso_prev is not None:
                                emit_o_mms(prev[0], pso_prev, prev[1], ib, jt)
                            if pso_self is not None:
                                emit_o_mms(g, pso_self, ex, ib, jt)
                        if pso_prev is not None:
                            evict_o_half(prev[0], pso_prev, ib, prev[2])
                        if pso_self is not None:
                            evict_o_half(g, pso_self, ib, sdr)
                            if g == H // 2 - 1:
                                pending_fin.append(finish_norm_half(g, sdr, ib))
                    if self_o and g == 4:
                        pending_fin.append(finish_norm_dma(g, sdr))
                    if prev is not None:
                        pending_fin.append(finish_norm_dma(prev[0], prev[2]))
                    prev = None if self_o else (g, ex, sdr)
                for _ in a1_rest:
                    pass

                # ---------------- Phase C: output projection ----------------
                # Shares PSUM slots with tag "pss" -- no pool barrier, so C
                # starts as soon as the last pair is normalized.
                psy_tags = ((psBs, "pss"), (psBs, "pss"), (psBo, "pso0"),
                            (psBo, "pso1"), (psA, "psA1"), (psA, "psA1"))
                ci = 0
                for oi, it in enumerate((4, 5, 6, 7, 0, 1, 2, 3)):
                    if it == 0:
                        # last pair's ib=0 normalize: emit after the second-
                        # token-half chains so its reciprocal has landed.
                        for f in pending_fin:
                            f()
                        pending_fin = []
                    o_tile = pb.tile([P, C], F32, tag="yout", bufs=3)
                    psys = []
                    for cb, cw in ((0, 512), (512, 256)):
                        cpool, ctag = psy_tags[ci % 6]
                        ci += 1
                        psys.append((cpool.tile([P, 512], F32, tag=ctag, name="psy",
                                                 bufs=2 if ctag in ("pss", "psA1") else 1), cb, cw))
                    # both column chains share the per-gk stationary: emit them
                    # adjacently so LDWEIGHTS loads once per gk and hides under
                    # the other chain's streaming matmul.
                    for gk in range(KT):
                        for psy, cb, cw in psys:
                            nc.tensor.matmul(
                                psy[:, 0:cw],
                                ot_sb[gk][:, it * P:(it + 1) * P],
                                wp_sb[gk][:, cb:cb + cw],
                                start=(gk == 0), stop=False)
                    for psy, cb, cw in psys:
                        nc.tensor.matmul(
                            psy[:, 0:cw],
                            ones_row[:, it * P:(it + 1) * P],
                            bp_sb[:, cb:cb + cw],
                            start=False, stop=True)
                    # evict on Scalar (idle after phase B) for the wide
                    # half, Vector for the narrow one -- C never waits.
                    for psy, cb, cw in psys:
                        if cb == 0:
                            nc.scalar.activation(
                                o_tile[:, 0:512], psy[:, 0:512],
                                mybir.ActivationFunctionType.Copy)
                        else:
                            nc.vector.tensor_copy(o_tile[:, cb:cb + cw], psy[:, 0:cw])
                    # writeback in partition-quarter chunks with full-width
                    # rows (3KB contiguous DRAM descriptors), round-robin over
                    # all three DMA-capable rings -- each ring drains its DMAs
                    # near-sequentially, so ring count bounds throughput.
                    for q in range(4):
                        ring = (nc.sync, nc.gpsimd, nc.scalar)[(4 * oi + q) % 3]
                        ring.dma_start(
                            out=out[it * P + q * 32:it * P + (q + 1) * 32, :],
                            in_=o_tile[q * 32:(q + 1) * 32, :])

            if debug:
                for t in range(12):
                    nc.sync.dma_start(out=dbg["qkT"][t * P:(t + 1) * P, :], in_=qk_sb[t][:, :])
                for t in range(IT):
                    nc.sync.dma_start(out=dbg["vaug"][t * P:(t + 1) * P, :], in_=vaug_sb[t][:, :])
                for g in range(H // 2):
                    nc.sync.dma_start(out=dbg["ot"][g * P:(g + 1) * P, :], in_=ot_sb[g][:, :])

    nc.compile()
    return nc


_CACHED_NC = {}


def get_compiled(debug: bool = False):
    if debug not in _CACHED_NC:
        _CACHED_NC[debug] = build_attention(debug)
    return _CACHED_NC[debug]


def prep_inputs(x, w_qkv, q_bias, v_bias, w_proj, b_proj):
    x = np.asarray(x, np.float32)
    B = x.shape[0]
    import ml_dtypes
    bf16 = ml_dtypes.bfloat16
    xT = np.ascontiguousarray(x.transpose(0, 2, 1)).astype(bf16)
    wqkvT = np.ascontiguousarray(np.asarray(w_qkv, np.float32).T).astype(bf16)
    wprojT = np.ascontiguousarray(np.asarray(w_proj, np.float32).T).astype(bf16)
    qkb = np.ascontiguousarray((np.asarray(q_bias, np.float32) * SCALE).reshape(NT_Q, P).T)
    vbr = np.ascontiguousarray(np.asarray(v_bias, np.float32).reshape(1, C)).astype(bf16)
    bpr = np.ascontiguousarray(np.asarray(b_proj, np.float32).reshape(1, C)).astype(bf16)
    return [
        {"xT": xT[i], "wqkvT": wqkvT, "wprojT": wprojT, "qkb": qkb, "vb": vbr, "bp": bpr}
        for i in range(B)
    ]




def kernel(**inputs):
    """Harness entrypoint: full inputs in, full output out (8-way data parallel)."""
    from concourse.bass_utils import run_bass_kernel_spmd

    nc = get_compiled(False)
    in_maps = prep_inputs(
        inputs["x"], inputs["w_qkv"], inputs["q_bias"], inputs["v_bias"],
        inputs["w_proj"], inputs["b_proj"])
    res = run_bass_kernel_spmd(nc, in_maps, core_ids=list(range(8)))
    return np.stack([res.results[i]["out"] for i in range(len(in_maps))])

